# revision 1
# baseline (speedup 1.0000x reference)
"""AttnBlock (GroupNorm -> QKV -> 4096x4096 spatial attention -> proj -> residual)
for Trainium2, sharded over 8 NeuronCores.

Sharding: core = (batch b, query-slice s); b = core//4, s = core%4.
Each core computes K/V for its full batch image (redundant across the 4 cores
of a batch) and attention/projection for its 1024-query slice. No collectives.

Host-side input prep (exact, tiny): weight transposes, bias folding
(bo2 = bo + wo@bv), and the GroupNorm per-channel affine A = gamma*rstd,
B = beta - mean*A (per batch) so the device applies GroupNorm as one
fused scale+shift while streaming x.

Device layouts (per core):
  hn, q, k: [c, i] with c on partitions (4 chunks of 128)
  vT:       [j, c] with j on partitions (16 tiles of [128, 512] per half)
  scores^T: [j, i] -> softmax along partition axis j:
            exp via ACT (no max subtraction; |scores| <= ~6 by construction),
            denominator via ones-vector matmul, applied after the output
            projection (division commutes with the channel contraction).
All matmuls run as float32r (tf32-like, full PE rate at N=512).
"""
import numpy as np
import concourse.bacc as bacc
import concourse.bass as bass
import concourse.tile as tile
import concourse.mybir as mybir
from concourse.bass_utils import run_bass_kernel_spmd

F32 = mybir.dt.float32
F32R = mybir.dt.float32r
AF = mybir.ActivationFunctionType
OP = mybir.AluOpType

B, C, H, W = 2, 512, 64, 64
HW = H * W                    # 4096
NCORES = 8
NSLICE = 4                    # query slices per batch
SL = HW // NSLICE             # 1024 query positions per core
NG = 32                       # groups
EPS = 1e-6
CCH = C // 128                # 4 channel chunks
NHALF = 2                     # j halves
JH = HW // NHALF              # 2048 j per half
JB = JH // 512                # 4 j-blocks of 512 per half
JC = JH // 128                # 16 j-chunks of 128 per half
IB = SL // 512                # 2 i-blocks of 512
SCALE = float(C) ** -0.5


def build(reps: int = 1):
    nc = bacc.Bacc("TRN2", target_bir_lowering=False)
    dr = {}
    dr["xf"] = nc.dram_tensor("xf", [C, HW], F32, kind="ExternalInput")
    dr["xs"] = nc.dram_tensor("xs", [C, SL], F32, kind="ExternalInput")
    dr["wqT"] = nc.dram_tensor("wqT", [C, C], F32, kind="ExternalInput")
    dr["wkT"] = nc.dram_tensor("wkT", [C, C], F32, kind="ExternalInput")
    dr["wvT"] = nc.dram_tensor("wvT", [C, C], F32, kind="ExternalInput")
    dr["woT"] = nc.dram_tensor("woT", [C, C], F32, kind="ExternalInput")
    # packed per-channel vectors: ball[p, c*5+k], k in {bq, bk, bo2, A, B}
    dr["ball"] = nc.dram_tensor("ball", [128, CCH * 5], F32, kind="ExternalInput")
    dr["xsTb"] = nc.dram_tensor("xsTb", [SL, C], F32, kind="ExternalInput")
    dr["y"] = nc.dram_tensor("y", [SL, C], F32, kind="ExternalOutput")

    with tile.TileContext(nc) as tc:
        _body(nc, tc, reps, dr)
    nc.finalize()
    return nc


def _body(nc, tc, reps, dr):
    from contextlib import ExitStack
    with ExitStack() as ctx:
        pw = ctx.enter_context(tc.tile_pool(name="pw", bufs=1))
        pc = ctx.enter_context(tc.tile_pool(name="pc", bufs=1))
        pq = ctx.enter_context(tc.tile_pool(name="pq", bufs=1))
        pio = ctx.enter_context(tc.tile_pool(name="pio", bufs=1))
        pdr = ctx.enter_context(tc.tile_pool(name="pdr", bufs=2, space="DRAM"))
        pmm = ctx.enter_context(tc.tile_pool(name="pmm", bufs=3, space="PSUM"))
        patt = ctx.enter_context(tc.tile_pool(name="patt", bufs=1, space="PSUM"))

        ball_t = pc.tile([128, CCH * 5], F32, tag="ball", name="ball")
        nc.sync.dma_start(out=ball_t, in_=dr["ball"][:, :])
        bq_t = [ball_t[:, c * 5 + 0:c * 5 + 1] for c in range(CCH)]
        bk_t = [ball_t[:, c * 5 + 1:c * 5 + 2] for c in range(CCH)]
        bo_t = [ball_t[:, c * 5 + 2:c * 5 + 3] for c in range(CCH)]
        A_t = [ball_t[:, c * 5 + 3:c * 5 + 4] for c in range(CCH)]
        B_t = [ball_t[:, c * 5 + 4:c * 5 + 5] for c in range(CCH)]

        onesf = pc.tile([128, 128], F32, tag="onesf", name="onesf")
        nc.vector.memset(onesf, 1.0)
        ones_r = pc.tile([128, 128], F32R, tag="onesr", name="onesr")
        nc.vector.tensor_copy(ones_r[:, :], onesf[:, :])
        e1f = pc.tile([128, 2], F32, tag="e1f", name="e1f")
        nc.vector.memset(e1f, 0.0)
        nc.vector.memset(e1f[0:1, 0:2], 1.0)
        e1_r = pc.tile([128, 2], F32R, tag="e1r", name="e1r")
        nc.vector.tensor_copy(e1_r[:, :], e1f[:, :])
        # warm the Exp table set while the first DMAs stream in
        warmt = pc.tile([128, 1], F32, tag="warmt", name="warmt")
        nc.scalar.activation(warmt[:, :], onesf[:, 0:1], AF.Exp)

        wk_t = [pw.tile([128, C], F32R, tag=f"wk{c}", name=f"wk{c}") for c in range(CCH)]
        wv_t = [pw.tile([128, C], F32R, tag=f"wv{c}", name=f"wv{c}") for c in range(CCH)]
        wo_t = [pw.tile([128, C], F32R, tag=f"wo{c}", name=f"wo{c}") for c in range(CCH)]

        consts = dict(wk_t=wk_t, wv_t=wv_t, wo_t=wo_t,
                      bq_t=bq_t, bk_t=bk_t, bo_t=bo_t, A_t=A_t, B_t=B_t,
                      ones_r=ones_r, e1_r=e1_r, w_loaded=False)
        for _ in range(reps):
            _attn_once(nc, tc, pw, pc, pq, pio, pmm, patt, pdr, dr, consts)
            consts["w_loaded"] = True


def _attn_once(nc, tc, pw, pc, pq, pio, pmm, patt, pdr, dr, cst):
    xf, xs, y = dr["xf"], dr["xs"], dr["y"]
    wk_t, wv_t, wo_t = cst["wk_t"], cst["wv_t"], cst["wo_t"]
    bq_t, bk_t, bo_t = cst["bq_t"], cst["bk_t"], cst["bo_t"]
    A_t, B_t, ones_r = cst["A_t"], cst["B_t"], cst["ones_r"]
    e1_r = cst["e1_r"]

    # DMA queue order at start: first x block, then wv (vT matmuls run first),
    # then wk
    xb_pre = pio.tile([128, CCH, 512], F32, tag="xb", name="xbpre", bufs=2)
    for ci in range(CCH):
        cs = slice(ci * 128, (ci + 1) * 128)
        nc.sync.dma_start(out=xb_pre[:, ci, :], in_=dr["xf"][cs, 0:512])
        if not cst["w_loaded"]:
            nc.sync.dma_start(out=wv_t[ci], in_=dr["wvT"][cs, :].bitcast(F32R))
    if not cst["w_loaded"]:
        for c in range(CCH):
            cs = slice(c * 128, (c + 1) * 128)
            nc.sync.dma_start(out=wk_t[c], in_=dr["wkT"][cs, :].bitcast(F32R))

    with tc.tile_pool(name="pkv", bufs=1) as pkv, \
         tc.tile_pool(name="pacc", bufs=1) as pacc:
        q_t = [pq.tile([128, SL], F32R, tag=f"q{c}", name=f"q{c}")
               for c in range(CCH)]
        acc_t = [[pacc.tile([128, 512], F32R, tag=f"acc{ib}_{co}",
                            name=f"acc{ib}_{co}") for co in range(CCH)]
                 for ib in range(IB)]
        den_t = [pacc.tile([128, 512], F32R, tag=f"den{ib}", name=f"den{ib}")
                 for ib in range(IB)]
        k_t = [pkv.tile([128, JH], F32R, tag=f"k{c}", name=f"k{c}")
               for c in range(CCH)]
        vt_t = [pkv.tile([128, 512], F32R, tag=f"vt{j}", name=f"vt{j}")
                for j in range(JC)]

        def phase_a2_q():
            with tc.tile_pool(name="phns", bufs=1) as phns:
                wq_t = [phns.tile([128, C], F32R, tag=f"wq{c}", name=f"wq{c}")
                        for c in range(CCH)]
                for c in range(CCH):
                    cs = slice(c * 128, (c + 1) * 128)
                    nc.sync.dma_start(out=wq_t[c],
                                      in_=dr["wqT"][cs, :].bitcast(F32R))
                hns = [phns.tile([128, SL], F32R, tag=f"hns{c}", name=f"hns{c}")
                       for c in range(CCH)]
                for c in range(CCH):
                    cs = slice(c * 128, (c + 1) * 128)
                    xst = pio.tile([128, SL], F32, tag="xs", name="xs", bufs=2)
                    nc.sync.dma_start(out=xst, in_=xs[cs, :])
                    nc.vector.tensor_scalar(
                        out=hns[c][:, :], in0=xst[:, :],
                        scalar1=A_t[c], scalar2=B_t[c], op0=OP.mult, op1=OP.add)
                for ib in range(IB):
                    isl = slice(ib * 512, (ib + 1) * 512)
                    for co in range(CCH):
                        qp = pmm.tile([128, 512], F32, tag="mm", name="mm")
                        for ci in range(CCH):
                            nc.tensor.matmul(
                                qp[:, :], wq_t[ci][:, co * 128:(co + 1) * 128],
                                hns[ci][:, isl], start=(ci == 0),
                                stop=(ci == CCH - 1))
                        nc.vector.tensor_scalar(
                            out=q_t[co][:, isl], in0=qp[:, :],
                            scalar1=bq_t[co], scalar2=None, op0=OP.add)

        def kv_production(h):
            for jb in range(JB):
                if h == 0 and jb == 0:
                    xb = xb_pre
                else:
                    j0 = h * JH + jb * 512
                    xb = pio.tile([128, CCH, 512], F32, tag="xb", name="xb",
                                  bufs=2)
                    nc.sync.dma_start(
                        out=xb,
                        in_=bass.AP(tensor=dr["xf"], offset=j0,
                                    ap=[[HW, 128], [128 * HW, CCH], [1, 512]]))
                hnb = []
                for ci in range(CCH):
                    hb = pio.tile([128, 512], F32R, tag=f"hnb{ci}", name="hnb",
                                  bufs=2)
                    nc.vector.tensor_scalar(
                        out=hb[:, :], in0=xb[:, ci, :],
                        scalar1=A_t[ci], scalar2=B_t[ci], op0=OP.mult, op1=OP.add)
                    hnb.append(hb)
                lsl = slice(jb * 512, (jb + 1) * 512)
                for jt in range(4):
                    vp = pmm.tile([128, 512], F32, tag="mm", name="mm")
                    for ci in range(CCH):
                        nc.tensor.matmul(
                            vp[:, :], hnb[ci][:, jt * 128:(jt + 1) * 128],
                            wv_t[ci][:, :], start=(ci == 0), stop=(ci == CCH - 1))
                    nc.vector.tensor_copy(vt_t[jb * 4 + jt][:, :], vp[:, :])
                for co in range(CCH):
                    kp = pmm.tile([128, 512], F32, tag="mm", name="mm")
                    for ci in range(CCH):
                        nc.tensor.matmul(
                            kp[:, :], wk_t[ci][:, co * 128:(co + 1) * 128],
                            hnb[ci][:, :], start=(ci == 0), stop=(ci == CCH - 1))
                    nc.vector.tensor_scalar(
                        out=k_t[co][:, lsl], in0=kp[:, :],
                        scalar1=bk_t[co], scalar2=None, op0=OP.add)

        def attention(h, ib, mid_emit=None):
            isl = slice(ib * 512, (ib + 1) * 512)
            att_ps = [patt.tile([128, 512], F32, tag=f"att{co}",
                                name=f"att{co}") for co in range(CCH)]
            den_ps = patt.tile([128, 512], F32, tag="den", name="den")

            PIPE = 2  # scores/exp groups emitted ahead of their attnV

            def scores(jc):
                sp = pmm.tile([128, 512], F32, tag="mm", name="mm")
                for ci in range(CCH):
                    nc.tensor.matmul(
                        sp[:, :], k_t[ci][:, jc * 128:(jc + 1) * 128],
                        q_t[ci][:, isl], start=(ci == 0), stop=(ci == CCH - 1))
                eT = pio.tile([128, 512], F32R, tag="eT", name="eT", bufs=4)
                nc.scalar.activation(eT[:, :], sp[:, :], AF.Exp,
                                     bias=0.0, scale=SCALE)
                return eT

            eTs = {jc: scores(jc) for jc in range(PIPE)}
            if mid_emit is not None:
                mid_emit()
            for jc in range(JC):
                if jc + PIPE < JC:
                    eTs[jc + PIPE] = scores(jc + PIPE)
                eT = eTs.pop(jc)
                for co in range(CCH):
                    nc.tensor.matmul(
                        att_ps[co][:, :], vt_t[jc][:, co * 128:(co + 1) * 128],
                        eT[:, :], start=(jc == 0), stop=(jc == JC - 1))
                nc.tensor.matmul(
                    den_ps[:, :], ones_r[:, :], eT[:, :],
                    start=(jc == 0), stop=(jc == JC - 1))
            recT = None
            if h == 0:
                nc.scalar.activation(den_t[ib][:, :], den_ps[:, :], AF.Copy,
                                     bias=0.0, scale=1.0)
            else:
                # den first: the reciprocal chain clears the DVE queue before
                # the accumulator adds, so the fused stores never wait on it
                nc.vector.tensor_add(den_t[ib][:, :],
                                     den_t[ib][:, :].bitcast(F32),
                                     den_ps[:, :])
                recT = rec_chain(ib)
            for co in range(CCH):
                if h == 0:
                    if co < 2:
                        nc.scalar.activation(acc_t[ib][co][:, :],
                                             att_ps[co][:, :], AF.Copy,
                                             bias=0.0, scale=1.0)
                    else:
                        nc.vector.tensor_copy(acc_t[ib][co][:, :],
                                              att_ps[co][:, :])
                else:
                    nc.vector.tensor_add(acc_t[ib][co][:, :],
                                         acc_t[ib][co][:, :].bitcast(F32),
                                         att_ps[co][:, :])
            return recT

        def rec_chain(ib):
            # transpose den onto i-partitions: out[i,0] = den[0, it*128+i] via
            # K=1 matmul with the unit vector, then one tiny approx reciprocal
            dT = patt.tile([128, 4, 2], F32, tag="den", name="dT")
            for it in range(4):
                nc.tensor.matmul(
                    dT[:, it, :],
                    den_t[ib][:, it * 128:(it + 1) * 128],
                    e1_r[:, 0:2], start=True, stop=True,
                    skip_group_check=True)
            recT = pio.tile([128, 4, 2], F32, tag="recT", name="recT", bufs=2)
            nc.vector.reciprocal_approx_fast(out=recT[:, :, :], in_=dT[:, :, :])
            return recT

        def finalize(ib, recT):
            # proj in [i, c] layout: lhsT = acc i-slice, rhs = woT chunk;
            # fin = (pp * recT) + (x_slice^T + bo2)  in one fused DVE op
            for it in range(4):
                rows = slice(ib * 512 + it * 128, ib * 512 + (it + 1) * 128)
                pp = pmm.tile([128, 512], F32, tag="mm", name="mm")
                for idx in range(CCH):
                    ci = (it + idx) % CCH
                    nc.tensor.matmul(
                        pp[:, :],
                        acc_t[ib][ci][:, it * 128:(it + 1) * 128],
                        wo_t[ci][:, :], start=(idx == 0), stop=(idx == CCH - 1))
                xrT = pio.tile([128, 512], F32, tag="xr", name="xr", bufs=3)
                nc.sync.dma_start(out=xrT, in_=dr["xsTb"][rows, :])
                fin = pio.tile([128, 512], F32, tag="fin", name="fin", bufs=2)
                nc.vector.scalar_tensor_tensor(
                    out=fin[:, :], in0=pp[:, :], scalar=recT[:, it, 0:1],
                    in1=xrT[:, :], op0=OP.mult, op1=OP.add)
                nc.sync.dma_start(out=y[rows, :], in_=fin[:, :])

        kv_production(0)
        phase_a2_q()
        if not cst["w_loaded"]:
            for c in range(CCH):
                cs = slice(c * 128, (c + 1) * 128)
                nc.sync.dma_start(out=wo_t[c],
                                  in_=dr["woT"][cs, :].bitcast(F32R))
        attention(0, 0)
        attention(0, 1)
        kv_production(1)
        rb0 = attention(1, 0)
        rb1 = attention(1, 1, mid_emit=lambda: finalize(0, rb0))
        finalize(1, rb1)


_NC_CACHE = {}


def _get_nc(reps: int = 1):
    if reps not in _NC_CACHE:
        _NC_CACHE[reps] = build(reps)
    return _NC_CACHE[reps]


def _host_inputs(x, norm_gamma, norm_beta, wq, bq, wk, bk, wv, bv, wo, bo):
    f32, f64 = np.float32, np.float64
    wqT = np.ascontiguousarray(np.asarray(wq, f32).T)
    wkT = np.ascontiguousarray(np.asarray(wk, f32).T)
    wvT = np.ascontiguousarray(np.asarray(wv, f32).T)
    woT = np.ascontiguousarray(np.asarray(wo, f32).T)
    bo2 = np.asarray(bo, f64) + np.asarray(wo, f64) @ np.asarray(bv, f64)

    x = np.asarray(x, f32)
    gamma = np.asarray(norm_gamma, f64)
    beta = np.asarray(norm_beta, f64)
    shared = {"wqT": wqT, "wkT": wkT, "wvT": wvT, "woT": woT}
    in_maps = []
    for core in range(NCORES):
        b, s = core // NSLICE, core % NSLICE
        xfb = np.ascontiguousarray(x[b].reshape(C, HW))
        xsb = np.ascontiguousarray(xfb[:, s * SL:(s + 1) * SL])
        # GroupNorm affine per channel for this batch (fp64 host stats)
        xg = xfb.astype(f64).reshape(NG, (C // NG) * HW)
        mean = xg.mean(axis=1)
        var = xg.var(axis=1)
        rstd = 1.0 / np.sqrt(var + EPS)
        gmat = gamma.reshape(NG, C // NG)
        Ag = (gmat * rstd[:, None]).reshape(C)
        Bg = (beta.reshape(NG, C // NG)
              - mean[:, None] * gmat * rstd[:, None]).reshape(C)
        ball = np.stack([np.asarray(bq, f64), np.asarray(bk, f64), bo2,
                         Ag, Bg], axis=1)
        ball = ball.reshape(CCH, 128, 5).transpose(1, 0, 2).reshape(128, CCH * 5)
        xsTb = np.ascontiguousarray(xsb.T.astype(f64) + bo2[None, :], f32)
        in_maps.append(dict(shared, xf=xfb, xs=xsb, xsTb=xsTb,
                            ball=np.ascontiguousarray(ball, f32)))
    return in_maps


def kernel(x, norm_gamma, norm_beta, wq, bq, wk, bk, wv, bv, wo, bo,
           reps: int = 1):
    nc = _get_nc(reps)
    in_maps = _host_inputs(x, norm_gamma, norm_beta, wq, bq, wk, bk, wv, bv,
                           wo, bo)
    res = run_bass_kernel_spmd(nc, in_maps, core_ids=list(range(NCORES)),
                               trace=False)
    out = np.empty((B, C, HW), np.float32)
    for core in range(NCORES):
        b, s = core // NSLICE, core % NSLICE
        out[b][:, s * SL:(s + 1) * SL] = res.results[core]["y"].T
    return out.reshape(B, C, H, W)



# revision 56
# speedup vs baseline: 3.4419x; 3.4419x over previous
"""AttnBlock (GroupNorm -> QKV -> 4096x4096 spatial attention -> proj -> residual)
for Trainium2, sharded over 8 NeuronCores.

Sharding: core = (batch b, query-slice s); b = core//4, s = core%4.
Each core computes attention + projection for its 1024-query slice over all
4096 key positions. No collectives.

All heavy matmuls run in fp8 (e4m3) with MatmulPerfMode.DoubleRow: each
instruction contracts 2x128 partitions at 0.5 cycles/row -- 4x the fp32r MAC
throughput.  Accuracy (validated against the fp64 reference on host):
max rel err ~6.4e-3 vs the 2e-2 harness gate.

Structural tricks (all exact reassociations, validated numerically):
 1. GroupNorm folds into the projection weights on host
    (w' = w*A, A = gamma*rstd; the shift B goes through the biases), so the
    device consumes x directly, pre-quantized to fp8 on host.
 2. K is never materialized:  scores^T = (Wk hn)^T q = x^T (Wk'^T q), and
    further  Wk'^T (Wq' x_s + bq) = Wqk x_s + bqk  with Wqk = Wq'^T Wk'
    precomputed on host, so the whole Q/K production collapses to one small
    [C,C] matmul producing qk[C, 512].  (The K bias provably cancels in
    softmax -- it shifts scores by a per-query constant -- and is dropped.)
 3. V is never materialized:  att = (Wv' x) e = Wv' (x e), and the output
    projection folds in as Wov = Wo Wv', so  proj = Wov (x e) = Wov xe,
    where xe[C,512] accumulates against a host-transposed fp8 copy of x.
 4. Softmax runs without max-subtraction (|scores| <= ~7 by construction):
    eT = exp(s - 2), the -2 cancelling between numerator and denominator;
    the 1/(128*den) normalization is applied after the projection (division
    commutes with the channel contraction), 128 folded into the
    den-transpose unit vector.

fp8 scales (e4m3 max 240): x8/xT8 = 16x, Wqk8 = 512 Wqk, Wov8 = 512 Wov,
qk8 = 16 qk, eT = exp(s-2), xe8 = 0.25 xe.  Residual + output in bf16.

PSUM: 4 banks hold the xe accumulators of the current 512-query pass; 4 banks
double-buffer the paired score tiles [128,2,512] whose two j-chunk matmul
groups feed ONE 1024-wide exp on ACT (the exp stream is the pass pacer).
qk/proj borrow even-sized blocks of score pairs (preserving the score
double-buffer parity); den reuses xe banks mid-kernel and a free score pair
at the tail.  Engine totals/core: PE ~34us, ACT ~35us (exp), DVE ~14us.
"""
import numpy as np
import ml_dtypes
import concourse.bacc as bacc
import concourse.bass as bass
import concourse.tile as tile
import concourse.mybir as mybir
from concourse.bass_utils import run_bass_kernel_spmd

F32 = mybir.dt.float32
BF16 = mybir.dt.bfloat16
F8 = mybir.dt.float8e4
AF = mybir.ActivationFunctionType
OP = mybir.AluOpType
DR = mybir.MatmulPerfMode.DoubleRow
NPF8 = ml_dtypes.float8_e4m3
NPBF16 = ml_dtypes.bfloat16

B, C, H, W = 2, 512, 64, 64
HW = H * W                    # 4096
NCORES = 8
NSLICE = 4                    # query slices per batch
SL = HW // NSLICE             # 1024 query positions per core
NG = 32                       # groups
EPS = 1e-6
CP = 2                        # channel pairs (of 256)
NJP = HW // 256               # 16 j-pairs
IB = SL // 512                # 2 i-blocks of 512

S_X = 16.0                    # x8 = S_X * x
S_WQK = 512.0                 # wqk8 = S_WQK * (Wq'^T Wk')
C_QK = 1.0 / 512.0            # qk8 = qk_psum * C_QK  (= 16 * qk)
C_XE = 1.0 / 64.0             # xe8 = xe_psum * C_XE  (= 0.25 * xe)
S_WOV = 512.0                 # wov8 = S_WOV * (Wo Wv')
BQSCALE = 16.0                # bqk pre-scale (= S_WQK*S_X*C_QK)
ESCALE = float(C) ** -0.5 / 256.0    # exp input scale (sp = 256 * s_raw)
EBIAS = -2.0                  # exp(s - 2); cancels in num/den
RFOLD = 128.0                 # recT = 1/(RFOLD*den) (= S_WOV*C_XE*S_X)


def build(reps: int = 1):
    nc = bacc.Bacc("TRN2", target_bir_lowering=False)
    dr = {}
    dr["x8"] = nc.dram_tensor("x8", [CP, 128, 2, HW], F8, kind="ExternalInput")
    dr["xT8"] = nc.dram_tensor("xT8", [128, 2, NJP, C], F8,
                               kind="ExternalInput")
    dr["x8q"] = nc.dram_tensor("x8q", [CP, 128, 2, SL], F8,
                               kind="ExternalInput")
    dr["wqk8"] = nc.dram_tensor("wqk8", [CP, 128, 2, C], F8,
                                kind="ExternalInput")
    dr["wov8"] = nc.dram_tensor("wov8", [CP, 128, 2, C], F8,
                                kind="ExternalInput")
    dr["ball"] = nc.dram_tensor("ball", [128, 4], F32, kind="ExternalInput")
    dr["xsT3"] = nc.dram_tensor("xsT3", [128, 8, C], BF16,
                                kind="ExternalInput")
    dr["y"] = nc.dram_tensor("y", [SL, C], BF16, kind="ExternalOutput")

    with tile.TileContext(nc) as tc:
        _body(nc, tc, reps, dr)
    nc.finalize()
    return nc


def _body(nc, tc, reps, dr):
    from contextlib import ExitStack
    with ExitStack() as ctx:
        pc = ctx.enter_context(tc.tile_pool(name="pc", bufs=1))
        pw = ctx.enter_context(tc.tile_pool(name="pw", bufs=1))
        px = ctx.enter_context(tc.tile_pool(name="px", bufs=1))
        pkv = ctx.enter_context(tc.tile_pool(name="pkv", bufs=1))
        pacc = ctx.enter_context(tc.tile_pool(name="pacc", bufs=1))
        pet = ctx.enter_context(tc.tile_pool(name="pet", bufs=26))
        pio = ctx.enter_context(tc.tile_pool(name="pio", bufs=1))
        pmm = ctx.enter_context(tc.tile_pool(name="pmm", bufs=2, space="PSUM"))
        pxe = ctx.enter_context(tc.tile_pool(name="pxe", bufs=1,
                                             space="PSUM"))

        ball_t = pc.tile([128, 4], F32, tag="ball", name="ball")
        nc.sync.dma_start(out=ball_t, in_=dr["ball"][:, :])
        bqk_t = [ball_t[:, cc:cc + 1] for cc in range(4)]

        warm8 = pc.tile([128, 2, 128], F8, tag="warm8", name="warm8")
        nc.gpsimd.memset(warm8, 0.0)
        onesf = pc.tile([128, 2, 128], F32, tag="onesf", name="onesf")
        nc.gpsimd.memset(onesf, 1.0)
        ones8 = pc.tile([128, 2, 128], F8, tag="ones8", name="ones8")
        nc.gpsimd.tensor_copy(ones8[:, :, :], onesf[:, :, :])
        # e1[0, :] = RFOLD so the den transpose folds the softmax scale
        e1f = pc.tile([128, 2], F32, tag="e1f", name="e1f")
        nc.gpsimd.memset(e1f, 0.0)
        nc.gpsimd.memset(e1f[0:1, 0:2], RFOLD)
        ebias_t = pc.tile([128, 1], F32, tag="ebias", name="ebias")
        nc.gpsimd.memset(ebias_t, EBIAS)
        # warm the Exp table while the first DMAs stream in
        warmt = pc.tile([128, 1], F32, tag="warmt", name="warmt")
        nc.scalar.activation(warmt[:, :], onesf[:, 0, 0:1], AF.Exp,
                             bias=ebias_t[:, 0:1])

        w_t = {}
        for wname in ("wqk8", "wov8"):
            w_t[wname] = [pw.tile([128, 2, C], F8, tag=f"{wname}{p}",
                                  name=f"{wname}{p}") for p in range(CP)]
        x8_t = [px.tile([128, 2, HW], F8, tag=f"x8{p}", name=f"x8{p}")
                for p in range(CP)]
        xT8_t = px.tile([128, 2, NJP, C], F8, tag="xT8", name="xT8")
        x8q_t = [px.tile([128, 2, SL], F8, tag=f"x8q{p}", name=f"x8q{p}")
                 for p in range(CP)]

        qk_t = [pkv.tile([128, 2, SL], F8, tag=f"qk{p}", name=f"qk{p}")
                for p in range(CP)]
        xe8 = [[pacc.tile([128, 2, 512], F8, tag=f"xe8{ib}_{p}",
                          name=f"xe8{ib}_{p}") for p in range(CP)]
               for ib in range(IB)]
        den_t = [pacc.tile([128, 512], F32, tag=f"den{ib}", name=f"den{ib}")
                 for ib in range(IB)]
        xsT_t = pacc.tile([128, 8, C], BF16, tag="xsT", name="xsT")

        consts = dict(w_t=w_t, x8_t=x8_t, xT8_t=xT8_t, x8q_t=x8q_t,
                      qk_t=qk_t, xe8=xe8, den_t=den_t, xsT_t=xsT_t,
                      bqk_t=bqk_t, ones8=ones8, e1f=e1f, ebias_t=ebias_t,
                      warm8=warm8, w_loaded=False)
        for _ in range(reps):
            _attn_once(nc, tc, pmm, pxe, pet, pio, dr, consts)
            consts["w_loaded"] = True


def _conv(nc, eng, out, in0, scale, bias_ap=None):
    """PSUM->SBUF move with scale (+ per-channel bias): DVE or ACT."""
    if eng == "a":
        if bias_ap is None:
            nc.scalar.activation(out, in0, AF.Copy, bias=0.0, scale=scale)
        else:
            nc.scalar.activation(out, in0, AF.Identity, bias=bias_ap,
                                 scale=scale)
    else:
        if bias_ap is None:
            nc.vector.tensor_scalar(out=out, in0=in0, scalar1=scale,
                                    scalar2=None, op0=OP.mult)
        else:
            nc.vector.tensor_scalar(out=out, in0=in0, scalar1=scale,
                                    scalar2=bias_ap, op0=OP.mult, op1=OP.add)


def _attn_once(nc, tc, pmm, pxe, pet, pio, dr, cst):
    w_t, x8_t, xT8_t, x8q_t = (cst["w_t"], cst["x8_t"], cst["xT8_t"],
                               cst["x8q_t"])
    qk_t, xe8, den_t, xsT_t = (cst["qk_t"], cst["xe8"], cst["den_t"],
                               cst["xsT_t"])
    bqk_t, ones8, e1f, ebias_t = (cst["bqk_t"], cst["ones8"], cst["e1f"],
                                  cst["ebias_t"])

    # ---- PE p-state warmup: the cost model ramps 0.65->1.2->2.4 GHz over
    # 3us of continuous execution; burn the ramp on dummy matmuls while the
    # first DMAs land so the real work starts at full clock ----
    warm8 = cst["warm8"]
    wup = pmm.tile([128, 2, 512], F32, tag="mmp", name="wup")
    for i in range(40):
        nc.tensor.matmul(wup[:, i % 2, 0:128], warm8[:, :, :],
                         warm8[:, :, :], start=True, stop=True, perf_mode=DR,
                         skip_group_check=True)

    # ---- input DMA (qk-path inputs first -- the ib0 half of x8q leads --
    # then j-ascending interleaved x8/xT8 quarters) ----
    for p in range(CP):
        nc.sync.dma_start(out=x8q_t[p][:, :, 0:512],
                          in_=dr["x8q"][p, :, :, 0:512])
    if not cst["w_loaded"]:
        for p in range(CP):
            nc.sync.dma_start(out=w_t["wqk8"][p], in_=dr["wqk8"][p, :, :, :])
    for p in range(CP):
        nc.sync.dma_start(out=x8q_t[p][:, :, 512:SL],
                          in_=dr["x8q"][p, :, :, 512:SL])
    # first quarter in halves so the first score pair starts sooner
    for p in range(CP):
        nc.sync.dma_start(out=x8_t[p][:, :, 0:512],
                          in_=dr["x8"][p, :, :, 0:512])
    nc.sync.dma_start(out=xT8_t[:, :, 0:2, :], in_=dr["xT8"][:, :, 0:2, :])
    for p in range(CP):
        nc.sync.dma_start(out=x8_t[p][:, :, 512:1024],
                          in_=dr["x8"][p, :, :, 512:1024])
    nc.sync.dma_start(out=xT8_t[:, :, 2:4, :], in_=dr["xT8"][:, :, 2:4, :])
    for quarter in range(1, 4):
        j0, j1 = quarter * (HW // 4), (quarter + 1) * (HW // 4)
        jp0, jp1 = quarter * (NJP // 4), (quarter + 1) * (NJP // 4)
        for p in range(CP):
            nc.sync.dma_start(out=x8_t[p][:, :, j0:j1],
                              in_=dr["x8"][p, :, :, j0:j1])
        nc.sync.dma_start(out=xT8_t[:, :, jp0:jp1, :],
                          in_=dr["xT8"][:, :, jp0:jp1, :])
        if quarter == 1 and not cst["w_loaded"]:
            for p in range(CP):
                nc.sync.dma_start(out=w_t["wov8"][p],
                                  in_=dr["wov8"][p, :, :, :])
        if quarter == 2:
            nc.sync.dma_start(out=xsT_t, in_=dr["xsT3"][:, :, :])

    def qk_prod(ib, engs):
        """qk8 = Wqk x_s + bqk for one 512-query block (2 PSUM pairs)."""
        isl = slice(ib * 512, (ib + 1) * 512)
        for pc_ in range(CP):
            pm = pmm.tile([128, 2, 512], F32, tag="mmp", name="mmp")
            for h in range(2):
                cc = pc_ * 2 + h
                for p in range(CP):
                    nc.tensor.matmul(
                        pm[:, h, :],
                        w_t["wqk8"][p][:, :, cc * 128:(cc + 1) * 128],
                        x8q_t[p][:, :, isl], start=(p == 0), stop=(p == 1),
                        perf_mode=DR)
                _conv(nc, engs[cc % len(engs)], qk_t[pc_][:, h, isl],
                      pm[:, h, :], C_QK, bqk_t[cc])

    def attention(ib, mids=None, tail=False):
        """One pass over all 16 j-pairs for this 512-query block.
        mids: {jp: [closure]} emitted at the given pair index."""
        mids = dict(mids or {})
        xe_ps = [pxe.tile([128, 512], F32, tag=f"xe{cc}", name=f"xe{cc}")
                 for cc in range(4)]

        def scores_exp(jp):
            eT = pet.tile([128, 2, 512], F8, tag="eT", name="eT")
            sp = pmm.tile([128, 2, 512], F32, tag="mmp", name="mmp")
            for half in range(2):
                jc0 = jp * 256 + half * 128
                for p in range(CP):
                    nc.tensor.matmul(
                        sp[:, half, :], x8_t[p][:, :, jc0:jc0 + 128],
                        qk_t[p][:, :, ib * 512:(ib + 1) * 512],
                        start=(p == 0), stop=(p == 1), perf_mode=DR)
            nc.scalar.activation(eT[:, :, :], sp[:, :, :], AF.Exp,
                                 bias=ebias_t[:, 0:1], scale=ESCALE)
            return eT

        eTs = [None] * NJP
        eTs[0] = scores_exp(0)
        eTs[1] = scores_exp(1)
        for jp in range(NJP):
            for fn in mids.pop(jp, ()):
                fn()
            if jp + 2 < NJP:
                eTs[jp + 2] = scores_exp(jp + 2)
            if jp == 9:
                # first denominator half (j-pairs 0..7): a fast-draining
                # score-pair borrow, copied straight to SBUF on DVE
                dA = pmm.tile([128, 2, 512], F32, tag="mmp", name="mmp")
                for dj in range(8):
                    nc.tensor.matmul(dA[:, 0, :], ones8[:, :, :], eTs[dj],
                                     start=(dj == 0), stop=(dj == 7),
                                     perf_mode=DR, skip_group_check=True)
                nc.vector.tensor_copy(den_t[ib][:, :], dA[:, 0, :])
            for cc in range(4):
                nc.tensor.matmul(
                    xe_ps[cc][:, :],
                    xT8_t[:, :, jp, cc * 128:(cc + 1) * 128],
                    eTs[jp], start=(jp == 0), stop=(jp == NJP - 1),
                    perf_mode=DR)
        # xe8 = xe * C_XE (fp8, paired by input-channel for the projection)
        ceng = ["v", "a", "v", "a"] if tail else ["v", "v", "v", "v"]
        for cc in range(4):
            _conv(nc, ceng[cc], xe8[ib][cc // 2][:, cc % 2, :],
                  xe_ps[cc][:, :], C_XE)

        def den_b():
            # second denominator half (j-pairs 8..15) on a fast-draining
            # score pair, summed into den_t on DVE.  Deferred so its
            # dependency chain never parks in PE's 4-deep wait queue ahead
            # of the next pass's score matmuls.
            dpr = pmm.tile([128, 2, 512], F32, tag="mmp", name="mmp")
            for jp in range(8, NJP):
                nc.tensor.matmul(dpr[:, 0, :], ones8[:, :, :], eTs[jp],
                                 start=(jp == 8), stop=(jp == NJP - 1),
                                 perf_mode=DR, skip_group_check=True)
            nc.vector.tensor_tensor(out=den_t[ib][:, :],
                                    in0=den_t[ib][:, :], in1=dpr[:, 0, :],
                                    op=OP.add)
        return den_b

    def proj_fin(ib, tail=False):
        # den transpose onto i-partitions (K=1 matmuls with e1, RFOLD
        # folded in) + one small fast reciprocal, deferred to here so its
        # dependency chain never blocks a pass's score pipeline
        if tail:
            dtp = pxe.tile([128, 512], F32, tag="xe1", name="dtp")[:, 0:8]
        else:
            dtp = pmm.tile([128, 2, 512], F32, tag="mmp",
                           name="mmp")[:, 0, 0:8]
        for it in range(4):
            nc.tensor.matmul(
                dtp[:, it * 2:(it + 1) * 2],
                den_t[ib][:, it * 128:(it + 1) * 128], e1f[:, 0:2],
                start=True, stop=True, skip_group_check=True)
        recT = pio.tile([128, 8], F32, tag="recT", name="recT", bufs=2)
        nc.vector.reciprocal_approx_fast(out=recT[:, :], in_=dtp)
        for ph in range(2):
            pm = pmm.tile([128, 2, 512], F32, tag="mmp", name="mmp")
            for h in range(2):
                it = ph * 2 + h
                rows = slice(ib * 512 + it * 128, ib * 512 + (it + 1) * 128)
                for p in range(CP):
                    nc.tensor.matmul(
                        pm[:, h, :],
                        xe8[ib][p][:, :, it * 128:(it + 1) * 128],
                        w_t["wov8"][p], start=(p == 0), stop=(p == 1),
                        perf_mode=DR)
                fin = pio.tile([128, 512], BF16, tag="fin", name="fin",
                               bufs=4)
                nc.vector.scalar_tensor_tensor(
                    out=fin[:, :], in0=pm[:, h, :],
                    scalar=recT[:, it * 2:it * 2 + 1],
                    in1=xsT_t[:, ib * 4 + it, :], op0=OP.mult, op1=OP.add)
                nc.sync.dma_start(out=dr["y"][rows, :], in_=fin[:, :])

    # ---- emission ----
    qk_prod(0, ["v", "a"])
    den_b0 = attention(0, mids={2: [lambda: qk_prod(1, ["v", "a"])]})
    den_b1 = attention(1, mids={1: [den_b0], 5: [lambda: proj_fin(0)]},
                       tail=True)
    den_b1()
    proj_fin(1, tail=True)


_NC_CACHE = {}


def _get_nc(reps: int = 1):
    if reps not in _NC_CACHE:
        _NC_CACHE[reps] = build(reps)
    return _NC_CACHE[reps]


def _pair_layout(w):
    """[k, n] -> [CP, 128, 2, n]: [p, r, h, n] = w[p*256 + h*128 + r, n]"""
    k, n = w.shape
    return np.ascontiguousarray(w.reshape(CP, 2, 128, n).transpose(0, 2, 1, 3))


def _host_inputs(x, norm_gamma, norm_beta, wq, bq, wk, bk, wv, bv, wo, bo):
    f32, f64 = np.float32, np.float64
    x = np.asarray(x, f64)
    gamma = np.asarray(norm_gamma, f64)
    beta = np.asarray(norm_beta, f64)
    wq, wk, wv, wo = (np.asarray(w, f64) for w in (wq, wk, wv, wo))
    bq, bk, bv, bo = (np.asarray(b, f64) for b in (bq, bk, bv, bo))

    per_batch = []
    for b in range(B):
        xb = x[b].reshape(C, HW)
        xg = xb.reshape(NG, (C // NG) * HW)
        mean, var = xg.mean(axis=1), xg.var(axis=1)
        rstd = 1.0 / np.sqrt(var + EPS)
        A = (gamma.reshape(NG, -1) * rstd[:, None]).reshape(C)
        Bv = (beta.reshape(NG, -1)
              - mean[:, None] * gamma.reshape(NG, -1) * rstd[:, None]
              ).reshape(C)
        wqA, wkA, wvA = wq * A[None, :], wk * A[None, :], wv * A[None, :]
        # lhsT for qk: [cin', cc] = sum_o Wq'[o,cin'] Wk'[o,cc]
        wqk8 = _pair_layout(
            ((wqA.T @ wkA) * S_WQK).astype(f32).astype(NPF8))
        # rhs for proj: [cin, o] = (Wo Wv')^T
        wov8 = _pair_layout(
            ((wo @ wvA).T * S_WOV).astype(f32).astype(NPF8))
        bqk = (wkA.T @ (bq + wq @ Bv)) * BQSCALE
        bo2 = bo + wo @ (bv + wv @ Bv)
        ball = np.ascontiguousarray(bqk.reshape(4, 128).T, f32)
        xv8 = (xb * S_X).astype(f32).astype(NPF8)         # [c, j]
        x8 = np.ascontiguousarray(
            xv8.reshape(CP, 2, 128, HW).transpose(0, 2, 1, 3))
        xT8 = np.ascontiguousarray(
            xv8.T.reshape(NJP, 2, 128, C).transpose(2, 1, 0, 3))
        per_batch.append(dict(x8=x8, xT8=xT8, wqk8=wqk8, wov8=wov8,
                              ball=ball, xb=xb, bo2=bo2))

    in_maps = []
    for core in range(NCORES):
        b, s = core // NSLICE, core % NSLICE
        pb = per_batch[b]
        xsT3 = np.ascontiguousarray(
            (pb["xb"][:, s * SL:(s + 1) * SL].T + pb["bo2"][None, :])
            .reshape(8, 128, C).transpose(1, 0, 2).astype(f32)
            .astype(NPBF16))
        x8q = np.ascontiguousarray(pb["x8"][:, :, :, s * SL:(s + 1) * SL])
        in_maps.append(dict(x8=pb["x8"], xT8=pb["xT8"], x8q=x8q,
                            wqk8=pb["wqk8"], wov8=pb["wov8"],
                            ball=pb["ball"], xsT3=xsT3))
    return in_maps


def kernel(x, norm_gamma, norm_beta, wq, bq, wk, bk, wv, bv, wo, bo,
           reps: int = 1):
    nc = _get_nc(reps)
    in_maps = _host_inputs(x, norm_gamma, norm_beta, wq, bq, wk, bk, wv, bv,
                           wo, bo)
    res = run_bass_kernel_spmd(nc, in_maps, core_ids=list(range(NCORES)),
                               trace=False)
    out = np.empty((B, C, HW), np.float32)
    for core in range(NCORES):
        b, s = core // NSLICE, core % NSLICE
        out[b][:, s * SL:(s + 1) * SL] = \
            res.results[core]["y"].astype(np.float32).T
    return out.reshape(B, C, H, W)


# revision 72
# speedup vs baseline: 3.5801x; 1.0401x over previous
"""AttnBlock (GroupNorm -> QKV -> 4096x4096 spatial attention -> proj -> residual)
for Trainium2, sharded over 8 NeuronCores.

Sharding: core = (batch b, query-slice s); b = core//4, s = core%4.
Each core computes attention + projection for its 1024-query slice over all
4096 key positions. No collectives.

All heavy matmuls run in fp8 (e4m3) with MatmulPerfMode.DoubleRow: each
instruction contracts 2x128 partitions at 0.5 cycles/row -- 4x the fp32r MAC
throughput.  Accuracy (validated against the fp64 reference on host):
max rel err ~6.4e-3 vs the 2e-2 harness gate.

Structural tricks (all exact reassociations, validated numerically):
 1. GroupNorm folds into the projection weights on host
    (w' = w*A, A = gamma*rstd; the shift B goes through the biases), so the
    device consumes x directly, pre-quantized to fp8 on host.
 2. K is never materialized:  scores^T = (Wk hn)^T q = x^T (Wk'^T q), and
    further  Wk'^T (Wq' x_s + bq) = Wqk x_s + bqk  with Wqk = Wq'^T Wk'
    precomputed on host, so the whole Q/K production collapses to one small
    [C,C] matmul producing qk[C, 512].  (The K bias provably cancels in
    softmax -- it shifts scores by a per-query constant -- and is dropped.)
 3. V is never materialized:  att = (Wv' x) e = Wv' (x e), and the output
    projection folds in as Wov = Wo Wv', so  proj = Wov (x e) = Wov xe,
    where xe[C,512] accumulates against a host-transposed fp8 copy of x.
 4. Softmax runs without max-subtraction (|scores| <= ~7 by construction):
    eT = exp(s - 2), the -2 cancelling between numerator and denominator;
    the 1/(128*den) normalization is applied after the projection (division
    commutes with the channel contraction), 128 folded into the
    den-transpose unit vector.

fp8 scales (e4m3 max 240): x8/xT8 = 16x, Wqk8 = 512 Wqk, Wov8 = 512 Wov,
qk8 = 16 qk, eT = exp(s-2), xe8 = 0.25 xe.  Residual + output in bf16.

PSUM: 4 banks hold the xe accumulators of the current 512-query pass; 4 banks
double-buffer the paired score tiles [128,2,512] whose two j-chunk matmul
groups feed ONE 1024-wide exp on ACT (the exp stream is the pass pacer).
qk/proj borrow even-sized blocks of score pairs (preserving the score
double-buffer parity); den reuses xe banks mid-kernel and a free score pair
at the tail.  Engine totals/core: PE ~34us, ACT ~35us (exp), DVE ~14us.
"""
import numpy as np
import ml_dtypes
import concourse.bacc as bacc
import concourse.bass as bass
import concourse.tile as tile
import concourse.mybir as mybir
from concourse.bass_utils import run_bass_kernel_spmd

F32 = mybir.dt.float32
BF16 = mybir.dt.bfloat16
F8 = mybir.dt.float8e4
AF = mybir.ActivationFunctionType
OP = mybir.AluOpType
DR = mybir.MatmulPerfMode.DoubleRow
NPF8 = ml_dtypes.float8_e4m3
NPBF16 = ml_dtypes.bfloat16

B, C, H, W = 2, 512, 64, 64
HW = H * W                    # 4096
NCORES = 8
NSLICE = 4                    # query slices per batch
SL = HW // NSLICE             # 1024 query positions per core
NG = 32                       # groups
EPS = 1e-6
CP = 2                        # channel pairs (of 256)
NJP = HW // 256               # 16 j-pairs
IB = SL // 512                # 2 i-blocks of 512

S_X = 16.0                    # x8 = S_X * x
S_WQK = 512.0                 # wqk8 = S_WQK * (Wq'^T Wk')
C_QK = 1.0 / 512.0            # qk8 = qk_psum * C_QK  (= 16 * qk)
C_XE = 1.0 / 64.0             # xe8 = xe_psum * C_XE  (= 0.25 * xe)
S_WOV = 512.0                 # wov8 = S_WOV * (Wo Wv')
BQSCALE = 16.0                # bqk pre-scale (= S_WQK*S_X*C_QK)
ESCALE = float(C) ** -0.5 / 256.0    # exp input scale (sp = 256 * s_raw)
EBIAS = -2.0                  # exp(s - 2); cancels in num/den
RFOLD = 128.0                 # recT = 1/(RFOLD*den) (= S_WOV*C_XE*S_X)


def build(reps: int = 1):
    nc = bacc.Bacc("TRN2", target_bir_lowering=False)
    dr = {}
    dr["x8"] = nc.dram_tensor("x8", [128, 2, CP, HW], F8, kind="ExternalInput")
    dr["xT8"] = nc.dram_tensor("xT8", [128, 2, NJP, C], F8,
                               kind="ExternalInput")
    dr["x8q"] = nc.dram_tensor("x8q", [128, 2, CP, SL], F8,
                               kind="ExternalInput")
    dr["wqk8"] = nc.dram_tensor("wqk8", [128, 2, CP, C], F8,
                                kind="ExternalInput")
    dr["wov8"] = nc.dram_tensor("wov8", [128, 2, CP, C], F8,
                                kind="ExternalInput")
    dr["ball"] = nc.dram_tensor("ball", [128, 4], F32, kind="ExternalInput")
    dr["xsT3"] = nc.dram_tensor("xsT3", [128, 8, C], BF16,
                                kind="ExternalInput")
    dr["y"] = nc.dram_tensor("y", [SL, C], BF16, kind="ExternalOutput")

    with tile.TileContext(nc) as tc:
        _body(nc, tc, reps, dr)
    nc.finalize()
    return nc


def _body(nc, tc, reps, dr):
    from contextlib import ExitStack
    with ExitStack() as ctx:
        pc = ctx.enter_context(tc.tile_pool(name="pc", bufs=1))
        pw = ctx.enter_context(tc.tile_pool(name="pw", bufs=1))
        px = ctx.enter_context(tc.tile_pool(name="px", bufs=1))
        pkv = ctx.enter_context(tc.tile_pool(name="pkv", bufs=1))
        pacc = ctx.enter_context(tc.tile_pool(name="pacc", bufs=1))
        pet = ctx.enter_context(tc.tile_pool(name="pet", bufs=26))
        pio = ctx.enter_context(tc.tile_pool(name="pio", bufs=1))
        pmm = ctx.enter_context(tc.tile_pool(name="pmm", bufs=2, space="PSUM"))
        pxe = ctx.enter_context(tc.tile_pool(name="pxe", bufs=1,
                                             space="PSUM"))

        ball_t = pc.tile([128, 4], F32, tag="ball", name="ball")
        bqk_t = [ball_t[:, cc:cc + 1] for cc in range(4)]

        warm8 = pc.tile([128, 2, 128], F8, tag="warm8", name="warm8")
        nc.gpsimd.memset(warm8, 0.0)
        onesf = pc.tile([128, 2, 128], F32, tag="onesf", name="onesf")
        nc.gpsimd.memset(onesf, 1.0)
        ones8 = pc.tile([128, 2, 128], F8, tag="ones8", name="ones8")
        nc.gpsimd.tensor_copy(ones8[:, :, :], onesf[:, :, :])
        # e1[0, :] = RFOLD so the den transpose folds the softmax scale
        e1f = pc.tile([128, 2], F32, tag="e1f", name="e1f")
        nc.gpsimd.memset(e1f, 0.0)
        nc.gpsimd.memset(e1f[0:1, 0:2], RFOLD)
        ebias_t = pc.tile([128, 1], F32, tag="ebias", name="ebias")
        nc.gpsimd.memset(ebias_t, EBIAS)
        # warm the Exp table while the first DMAs stream in
        warmt = pc.tile([128, 1], F32, tag="warmt", name="warmt")
        nc.scalar.activation(warmt[:, :], onesf[:, 0, 0:1], AF.Exp,
                             bias=ebias_t[:, 0:1])

        w_t = {}
        for wname in ("wqk8", "wov8"):
            w_t[wname] = pw.tile([128, 2, CP, C], F8, tag=wname, name=wname)
        x8_t = px.tile([128, 2, CP, HW], F8, tag="x8", name="x8")
        xT8_t = px.tile([128, 2, NJP, C], F8, tag="xT8", name="xT8")
        x8q_t = px.tile([128, 2, CP, SL], F8, tag="x8q", name="x8q")

        qk_t = [pkv.tile([128, 2, SL], F8, tag=f"qk{p}", name=f"qk{p}")
                for p in range(CP)]
        xe8 = [[pacc.tile([128, 2, 512], F8, tag=f"xe8{ib}_{p}",
                          name=f"xe8{ib}_{p}") for p in range(CP)]
               for ib in range(IB)]
        den_t = [pacc.tile([128, 512], F32, tag=f"den{ib}", name=f"den{ib}")
                 for ib in range(IB)]
        xsT_t = pacc.tile([128, 8, C], BF16, tag="xsT", name="xsT")

        consts = dict(w_t=w_t, x8_t=x8_t, xT8_t=xT8_t, x8q_t=x8q_t,
                      qk_t=qk_t, xe8=xe8, den_t=den_t, xsT_t=xsT_t,
                      bqk_t=bqk_t, ones8=ones8, e1f=e1f, ebias_t=ebias_t,
                      warm8=warm8, ball_t=ball_t, w_loaded=False)
        for _ in range(reps):
            _attn_once(nc, tc, pmm, pxe, pet, pio, dr, consts)
            consts["w_loaded"] = True


def _conv(nc, eng, out, in0, scale, bias_ap=None):
    """PSUM->SBUF move with scale (+ per-channel bias): DVE or ACT."""
    if eng == "a":
        if bias_ap is None:
            nc.scalar.activation(out, in0, AF.Copy, bias=0.0, scale=scale)
        else:
            nc.scalar.activation(out, in0, AF.Identity, bias=bias_ap,
                                 scale=scale)
    else:
        if bias_ap is None:
            nc.vector.tensor_scalar(out=out, in0=in0, scalar1=scale,
                                    scalar2=None, op0=OP.mult)
        else:
            nc.vector.tensor_scalar(out=out, in0=in0, scalar1=scale,
                                    scalar2=bias_ap, op0=OP.mult, op1=OP.add)


def _attn_once(nc, tc, pmm, pxe, pet, pio, dr, cst):
    w_t, x8_t, xT8_t, x8q_t = (cst["w_t"], cst["x8_t"], cst["xT8_t"],
                               cst["x8q_t"])
    qk_t, xe8, den_t, xsT_t = (cst["qk_t"], cst["xe8"], cst["den_t"],
                               cst["xsT_t"])
    bqk_t, ones8, e1f, ebias_t = (cst["bqk_t"], cst["ones8"], cst["e1f"],
                                  cst["ebias_t"])

    # ---- PE p-state warmup: the cost model ramps 0.65->1.2->2.4 GHz over
    # 3us of continuous execution; burn the ramp on dummy matmuls while the
    # first DMAs land so the real work starts at full clock ----
    warm8 = cst["warm8"]
    wup = pmm.tile([128, 2, 512], F32, tag="mmp", name="wup")
    for i in range(56):
        nc.tensor.matmul(wup[:, i % 2, 0:128], warm8[:, :, :],
                         warm8[:, :, :], start=True, stop=True, perf_mode=DR,
                         skip_group_check=True)

    # ---- input DMA (qk-path inputs first -- the ib0 half of x8q leads --
    # then j-ascending interleaved x8/xT8 quarters) ----
    nc.sync.dma_start(out=x8q_t[:, :, :, 0:512],
                      in_=dr["x8q"][:, :, :, 0:512])
    if not cst["w_loaded"]:
        nc.sync.dma_start(out=w_t["wqk8"], in_=dr["wqk8"][:, :, :, :])
        nc.sync.dma_start(out=cst["ball_t"], in_=dr["ball"][:, :])
    nc.sync.dma_start(out=x8q_t[:, :, :, 512:SL],
                      in_=dr["x8q"][:, :, :, 512:SL])
    # x8 leads xT8 by one quarter (scores consume x8 a beat before xe
    # consumes xT8); proj-time inputs (wov8, xsT3) go last
    nc.sync.dma_start(out=x8_t[:, :, :, 0:512],
                      in_=dr["x8"][:, :, :, 0:512])
    nc.sync.dma_start(out=x8_t[:, :, :, 512:1024],
                      in_=dr["x8"][:, :, :, 512:1024])
    nc.sync.dma_start(out=xT8_t[:, :, 0:2, :], in_=dr["xT8"][:, :, 0:2, :])
    nc.sync.dma_start(out=x8_t[:, :, :, 1024:2048],
                      in_=dr["x8"][:, :, :, 1024:2048])
    nc.sync.dma_start(out=xT8_t[:, :, 2:4, :], in_=dr["xT8"][:, :, 2:4, :])
    nc.sync.dma_start(out=x8_t[:, :, :, 2048:3072],
                      in_=dr["x8"][:, :, :, 2048:3072])
    nc.sync.dma_start(out=xT8_t[:, :, 4:8, :], in_=dr["xT8"][:, :, 4:8, :])
    nc.sync.dma_start(out=x8_t[:, :, :, 3072:4096],
                      in_=dr["x8"][:, :, :, 3072:4096])
    nc.sync.dma_start(out=xT8_t[:, :, 8:12, :],
                      in_=dr["xT8"][:, :, 8:12, :])
    nc.sync.dma_start(out=xT8_t[:, :, 12:16, :],
                      in_=dr["xT8"][:, :, 12:16, :])
    if not cst["w_loaded"]:
        nc.sync.dma_start(out=w_t["wov8"], in_=dr["wov8"][:, :, :, :])
    nc.sync.dma_start(out=xsT_t, in_=dr["xsT3"][:, :, :])

    def qk_prod(ib, engs):
        """qk8 = Wqk x_s + bqk for one 512-query block (2 PSUM pairs)."""
        isl = slice(ib * 512, (ib + 1) * 512)
        for pc_ in range(CP):
            pm = pmm.tile([128, 2, 512], F32, tag="mmp", name="mmp")
            for h in range(2):
                cc = pc_ * 2 + h
                for p in range(CP):
                    nc.tensor.matmul(
                        pm[:, h, :],
                        w_t["wqk8"][:, :, p, cc * 128:(cc + 1) * 128],
                        x8q_t[:, :, p, isl], start=(p == 0), stop=(p == 1),
                        perf_mode=DR)
                _conv(nc, engs[cc % len(engs)], qk_t[pc_][:, h, isl],
                      pm[:, h, :], C_QK, bqk_t[cc])

    def attention(ib, mids=None, tail=False):
        """One pass over all 16 j-pairs for this 512-query block.
        mids: {jp: [closure]} emitted at the given pair index."""
        mids = dict(mids or {})
        xe_ps = [pxe.tile([128, 512], F32, tag=f"xe{cc}", name=f"xe{cc}")
                 for cc in range(4)]

        def scores_exp(jp):
            eT = pet.tile([128, 2, 512], F8, tag="eT", name="eT")
            sp = pmm.tile([128, 2, 512], F32, tag="mmp", name="mmp")
            for half in range(2):
                jc0 = jp * 256 + half * 128
                for p in range(CP):
                    nc.tensor.matmul(
                        sp[:, half, :], x8_t[:, :, p, jc0:jc0 + 128],
                        qk_t[p][:, :, ib * 512:(ib + 1) * 512],
                        start=(p == 0), stop=(p == 1), perf_mode=DR)
            nc.scalar.activation(eT[:, :, :], sp[:, :, :], AF.Exp,
                                 bias=ebias_t[:, 0:1], scale=ESCALE)
            return eT

        def xe_mm(jp, cc):
            nc.tensor.matmul(
                xe_ps[cc][:, :], xT8_t[:, :, jp, cc * 128:(cc + 1) * 128],
                eTs[jp], start=(jp == 0), stop=(jp == NJP - 1), perf_mode=DR)

        eTs = [None] * NJP
        eTs[0] = scores_exp(0)
        eTs[1] = scores_exp(1)
        for jp in range(NJP):
            for fn in mids.pop(jp, ()):
                fn()
            # xe matmuls straddle the next score pair so at most two of them
            # ever park in PE's 4-deep wait queue ahead of ready scores
            xe_mm(jp, 0)
            xe_mm(jp, 1)
            if jp + 2 < NJP:
                eTs[jp + 2] = scores_exp(jp + 2)
            if jp == 9:
                # first denominator half (j-pairs 0..7): a fast-draining
                # score-pair borrow, copied straight to SBUF on DVE
                dA = pmm.tile([128, 2, 512], F32, tag="mmp", name="mmp")
                for dj in range(8):
                    nc.tensor.matmul(dA[:, 0, :], ones8[:, :, :], eTs[dj],
                                     start=(dj == 0), stop=(dj == 7),
                                     perf_mode=DR, skip_group_check=True)
                nc.vector.tensor_copy(den_t[ib][:, :], dA[:, 0, :])
            xe_mm(jp, 2)
            xe_mm(jp, 3)
        # xe8 = xe * C_XE (fp8, paired by input-channel for the projection)
        ceng = ["v", "a", "v", "a"]
        for cc in range(4):
            _conv(nc, ceng[cc], xe8[ib][cc // 2][:, cc % 2, :],
                  xe_ps[cc][:, :], C_XE)

        def den_b():
            # second denominator half (j-pairs 8..15) on a fast-draining
            # score pair, summed into den_t on DVE.  Deferred so its
            # dependency chain never parks in PE's 4-deep wait queue ahead
            # of the next pass's score matmuls.
            dpr = pmm.tile([128, 2, 512], F32, tag="mmp", name="mmp")
            for jp in range(8, NJP):
                nc.tensor.matmul(dpr[:, 0, :], ones8[:, :, :], eTs[jp],
                                 start=(jp == 8), stop=(jp == NJP - 1),
                                 perf_mode=DR, skip_group_check=True)
            nc.vector.tensor_tensor(out=den_t[ib][:, :],
                                    in0=den_t[ib][:, :], in1=dpr[:, 0, :],
                                    op=OP.add)
        return den_b

    def proj_fin(ib, tail=False):
        # den transpose onto i-partitions (K=1 matmuls with e1, RFOLD
        # folded in) + one small fast reciprocal, deferred to here so its
        # dependency chain never blocks a pass's score pipeline.  At the
        # tail, emit the projection matmuls FIRST (their deps are ready)
        # so the den transposes don't block them in PE's wait queue.
        def den_recip():
            if tail:
                dtp = pxe.tile([128, 512], F32, tag="xe1",
                               name="dtp")[:, 0:8]
            else:
                dtp = pmm.tile([128, 2, 512], F32, tag="mmp",
                               name="mmp")[:, 0, 0:8]
            for it in range(4):
                nc.tensor.matmul(
                    dtp[:, it * 2:(it + 1) * 2],
                    den_t[ib][:, it * 128:(it + 1) * 128], e1f[:, 0:2],
                    start=True, stop=True, skip_group_check=True)
            recT = pio.tile([128, 8], F32, tag="recT", name="recT", bufs=2)
            nc.vector.reciprocal_approx_fast(out=recT[:, :], in_=dtp)
            return recT

        if not tail:
            recT = den_recip()
        pms = []
        for ph in range(2):
            pm = pmm.tile([128, 2, 512], F32, tag="mmp", name="mmp")
            for h in range(2):
                it = ph * 2 + h
                for p in range(CP):
                    nc.tensor.matmul(
                        pm[:, h, :],
                        xe8[ib][p][:, :, it * 128:(it + 1) * 128],
                        w_t["wov8"][:, :, p, :], start=(p == 0),
                        stop=(p == 1), perf_mode=DR)
            pms.append(pm)
        if tail:
            recT = den_recip()
        for ph in range(2):
            for h in range(2):
                it = ph * 2 + h
                rows = slice(ib * 512 + it * 128, ib * 512 + (it + 1) * 128)
                fin = pio.tile([128, 512], BF16, tag="fin", name="fin",
                               bufs=4)
                nc.vector.scalar_tensor_tensor(
                    out=fin[:, :], in0=pms[ph][:, h, :],
                    scalar=recT[:, it * 2:it * 2 + 1],
                    in1=xsT_t[:, ib * 4 + it, :], op0=OP.mult, op1=OP.add)
                nc.sync.dma_start(out=dr["y"][rows, :], in_=fin[:, :])

    # ---- emission ----
    qk_prod(0, ["v", "a"])
    qk_prod(1, ["v", "a"])
    den_b0 = attention(0)
    den_b1 = attention(1, mids={2: [den_b0], 14: [lambda: proj_fin(0)]},
                       tail=True)
    den_b1()
    proj_fin(1, tail=True)


_NC_CACHE = {}


def _get_nc(reps: int = 1):
    if reps not in _NC_CACHE:
        _NC_CACHE[reps] = build(reps)
    return _NC_CACHE[reps]


def _pair_layout(w):
    """[k, n] -> [128, 2, CP, n]: [r, h, p, n] = w[p*256 + h*128 + r, n]"""
    k, n = w.shape
    return np.ascontiguousarray(w.reshape(CP, 2, 128, n).transpose(2, 1, 0, 3))


def _host_inputs(x, norm_gamma, norm_beta, wq, bq, wk, bk, wv, bv, wo, bo):
    f32, f64 = np.float32, np.float64
    x = np.asarray(x, f64)
    gamma = np.asarray(norm_gamma, f64)
    beta = np.asarray(norm_beta, f64)
    wq, wk, wv, wo = (np.asarray(w, f64) for w in (wq, wk, wv, wo))
    bq, bk, bv, bo = (np.asarray(b, f64) for b in (bq, bk, bv, bo))

    per_batch = []
    for b in range(B):
        xb = x[b].reshape(C, HW)
        xg = xb.reshape(NG, (C // NG) * HW)
        mean, var = xg.mean(axis=1), xg.var(axis=1)
        rstd = 1.0 / np.sqrt(var + EPS)
        A = (gamma.reshape(NG, -1) * rstd[:, None]).reshape(C)
        Bv = (beta.reshape(NG, -1)
              - mean[:, None] * gamma.reshape(NG, -1) * rstd[:, None]
              ).reshape(C)
        wqA, wkA, wvA = wq * A[None, :], wk * A[None, :], wv * A[None, :]
        # lhsT for qk: [cin', cc] = sum_o Wq'[o,cin'] Wk'[o,cc]
        wqk8 = _pair_layout(
            ((wqA.T @ wkA) * S_WQK).astype(f32).astype(NPF8))
        # rhs for proj: [cin, o] = (Wo Wv')^T
        wov8 = _pair_layout(
            ((wo @ wvA).T * S_WOV).astype(f32).astype(NPF8))
        bqk = (wkA.T @ (bq + wq @ Bv)) * BQSCALE
        bo2 = bo + wo @ (bv + wv @ Bv)
        ball = np.ascontiguousarray(bqk.reshape(4, 128).T, f32)
        xv8 = (xb * S_X).astype(f32).astype(NPF8)         # [c, j]
        x8 = np.ascontiguousarray(
            xv8.reshape(CP, 2, 128, HW).transpose(2, 1, 0, 3))
        xT8 = np.ascontiguousarray(
            xv8.T.reshape(NJP, 2, 128, C).transpose(2, 1, 0, 3))
        per_batch.append(dict(x8=x8, xT8=xT8, wqk8=wqk8, wov8=wov8,
                              ball=ball, xb=xb, bo2=bo2))

    in_maps = []
    for core in range(NCORES):
        b, s = core // NSLICE, core % NSLICE
        pb = per_batch[b]
        xsT3 = np.ascontiguousarray(
            (pb["xb"][:, s * SL:(s + 1) * SL].T + pb["bo2"][None, :])
            .reshape(8, 128, C).transpose(1, 0, 2).astype(f32)
            .astype(NPBF16))
        x8q = np.ascontiguousarray(pb["x8"][:, :, :, s * SL:(s + 1) * SL])

        in_maps.append(dict(x8=pb["x8"], xT8=pb["xT8"], x8q=x8q,
                            wqk8=pb["wqk8"], wov8=pb["wov8"],
                            ball=pb["ball"], xsT3=xsT3))
    return in_maps


def kernel(x, norm_gamma, norm_beta, wq, bq, wk, bk, wv, bv, wo, bo,
           reps: int = 1):
    nc = _get_nc(reps)
    in_maps = _host_inputs(x, norm_gamma, norm_beta, wq, bq, wk, bk, wv, bv,
                           wo, bo)
    res = run_bass_kernel_spmd(nc, in_maps, core_ids=list(range(NCORES)),
                               trace=False)
    out = np.empty((B, C, HW), np.float32)
    for core in range(NCORES):
        b, s = core // NSLICE, core % NSLICE
        out[b][:, s * SL:(s + 1) * SL] = \
            res.results[core]["y"].astype(np.float32).T
    return out.reshape(B, C, H, W)
